# revision 2
# baseline (speedup 1.0000x reference)
"""Trainium2 Bass kernel: ring attention forward == full softmax attention.

The reference's ring decomposition with the sigmoid/logsigmoid LSE merge is
mathematically exact online softmax, so the output equals plain (non-causal)
multi-head attention over the full sequence:

    out[b,q,h,:] = softmax(Q[b,q,h,:] @ K[b,:,h,:].T / sqrt(D)) @ V[b,:,h,:]

Shapes: B=1, S=4096, H=16, D=128, fp32. ring_size only affects the reference's
chunking, not the result, so it is ignored here.

Sharding: 2 heads per NeuronCore (16 heads / 8 cores), fully independent --
no cross-core communication needed (Ulysses-style head sharding).

Device algorithm per head (flash-style, transposed-scores orientation).
Per 1024-wide q superblock, the 32 k-tiles are processed in groups of
(A: 2 k-tiles -> one [128,2048] psum score tile, B: 1 k-tile -> [128,1024]):

      scores_T[k,q] = K_tile^T-layout @ Q^T-layout    (PE, bf16, psum fp32)
      P_T = exp(scores_T * scale)                     (ACT FD=2048/1024, bf16)
        ... or for N_DVE_A of the A-tiles: a one-instruction Schraudolph
        fast-exp on the DVE (int16 bit trick -> bf16), offloading the
        saturated ACT engine
      out_T[d,q] += V_tile^T @ P_T                    (PE, accumulate psum)
      tree-sum of P_T tiles                           (DVE, bf16 2x mode)
  drain: out_T psum -> sbuf (DVE) -> DRAM; two tree roots -> DRAM raw.

Normalization (out/l) and the [d,q]->[q,d] transpose happen on the host
during unsharding: l = per-q partition-sum of the two roots. This removes
all PE transposes and the psum-resident l machinery from the device.

ACT instruction overhead amortization: FD=2048 activations cover 2 k-tiles
each (PSUM: 4-bank A tile + 2-bank B tile + 2-bank out accumulator = 8).
"""

import numpy as np
import ml_dtypes
from contextlib import ExitStack

import concourse.bass as bass
import concourse.bacc as bacc
import concourse.mybir as mybir
import concourse.tile as tile
from concourse.bass_utils import run_bass_kernel_spmd

B, S, H, D = 1, 4096, 16, 128
N_CORES = 8
HPC = H // N_CORES          # heads per core
SB = 1024                   # q superblock width
NSB = S // SB
NKT = S // 128              # 32 k-tiles of 128 keys
SCALE = float(1.0 / np.sqrt(D))
BF16 = mybir.dt.bfloat16
FP32 = mybir.dt.float32
I16 = mybir.dt.int16

# Groups per superblock: 10x (A: kt 3g,3g+1 | B: kt 3g+2) + 1x (A: kt 30,31)
GROUPS = [(3 * g, 3 * g + 1, 3 * g + 2) for g in range(10)] + [(30, 31, None)]

# How many of the 11 A score tiles per superblock take the DVE fast-exp
# path instead of ACT (0 = exact exp everywhere). Offloaded tiles use the
# Schraudolph bit trick: bf16(exp(x)) ~= bitcast_bf16(int16(x*A + B)).
N_DVE_A = 0
SCHRAUD_SIGMA = 0.06
SCHRAUD_A = SCALE * 128.0 * np.log2(np.e)
SCHRAUD_B = 128.0 * (127.0 - SCHRAUD_SIGMA)

_CACHE = {}


def _build():
    nc = bacc.Bacc("TRN2", target_bir_lowering=False, debug=False)
    # Inputs per core (host pre-arranged, bf16):
    #   qt/kt: [head, d, s]  (transposed layout, d on partitions)
    #   vp:    [head, p, t*128+c] where vp[h, p, 128t+c] = V[128t+p, c]
    qt_d = nc.dram_tensor("qt", [HPC, 128, S], BF16, kind="ExternalInput")
    kt_d = nc.dram_tensor("kt", [HPC, 128, S], BF16, kind="ExternalInput")
    vp_d = nc.dram_tensor("vp", [HPC, 128, S], BF16, kind="ExternalInput")
    # Outputs: unnormalized out in [d, q] layout + raw tree roots (host
    # finishes l = partition-sum of roots, then out/l and transpose).
    o_d = nc.dram_tensor("o", [HPC, NSB, 128, SB], FP32, kind="ExternalOutput")
    r_d = nc.dram_tensor("r", [HPC, NSB, 2, 128, SB], BF16, kind="ExternalOutput")

    with ExitStack() as ctx:
        tc = ctx.enter_context(tile.TileContext(nc))

        qkv = ctx.enter_context(tc.tile_pool(name="qkv", bufs=2))
        ptap = ctx.enter_context(tc.tile_pool(name="ptap", bufs=3))
        ptbp = ctx.enter_context(tc.tile_pool(name="ptbp", bufs=3))
        trap = ctx.enter_context(tc.tile_pool(name="trap", bufs=2))
        trbp = ctx.enter_context(tc.tile_pool(name="trbp", bufs=2))
        drainp = ctx.enter_context(tc.tile_pool(name="drainp", bufs=2))

        # PSUM budget: 8 banks of [128, 512 fp32].
        scap = ctx.enter_context(tc.tile_pool(name="scap", bufs=1, space="PSUM"))  # 4 banks
        scbp = ctx.enter_context(tc.tile_pool(name="scbp", bufs=1, space="PSUM"))  # 2 banks
        otp = ctx.enter_context(tc.tile_pool(name="otp", bufs=1, space="PSUM"))    # 2 banks

        for h in range(HPC):
            # Chunked loads so the first QK can start before full tensors land.
            qt_s = qkv.tile([128, S], BF16, name=f"qt{h}", tag="qt")
            kt_s = qkv.tile([128, S], BF16, name=f"kt{h}", tag="kt")
            v_s = qkv.tile([128, S], BF16, name=f"v{h}", tag="v")
            for ch in range(4):
                cs = slice(ch * (S // 4), (ch + 1) * (S // 4))
                nc.sync.dma_start(kt_s[:, cs], kt_d[h][:, cs])
                nc.sync.dma_start(qt_s[:, cs], qt_d[h][:, cs])
                nc.sync.dma_start(v_s[:, cs], vp_d[h][:, cs])

            for sb in range(NSB):
                q0 = sb * SB
                ot = otp.tile([128, SB], FP32, name=f"ot_{h}_{sb}", tag="ot")

                # Binary tree-sums of the P_T tiles on the DVE (bf16, 2x
                # mode). A-tiles tree at [128,2048], B-tiles at [128,1024];
                # the two roots go to DRAM raw and the host finishes the
                # 128-partition sum for l.
                trees = {"A": {}, "B": {}}
                treectr = [0]

                def feed(which, t, level=0):
                    tree = trees[which]
                    pool = trap if which == "A" else trbp
                    w = 2 * SB if which == "A" else SB
                    while level in tree:
                        prev = tree.pop(level)
                        treectr[0] += 1
                        nt = pool.tile(
                            [128, w], BF16,
                            name=f"tr{which}_{h}_{sb}_{level}_{treectr[0]}",
                            tag=f"tree{which}{level}",
                        )
                        nc.vector.tensor_add(nt, prev, t)
                        t = nt
                        level += 1
                    tree[level] = t

                def collapse(which):
                    tree = trees[which]
                    pool = trap if which == "A" else trbp
                    w = 2 * SB if which == "A" else SB
                    levels = sorted(tree)
                    t = tree.pop(levels[0])
                    for lv in levels[1:]:
                        prev = tree.pop(lv)
                        treectr[0] += 1
                        nt = pool.tile(
                            [128, w], BF16,
                            name=f"trc{which}_{h}_{sb}_{treectr[0]}",
                            tag=f"tree{which}c",
                        )
                        nc.vector.tensor_add(nt, prev, t)
                        t = nt
                    return t

                def qk(sc_dst, col0, j):
                    for qs in range(2):
                        nc.tensor.matmul(
                            sc_dst[:, col0 + qs * 512: col0 + (qs + 1) * 512],
                            kt_s[:, j * 128:(j + 1) * 128],
                            qt_s[:, q0 + qs * 512: q0 + (qs + 1) * 512],
                            start=True, stop=True,
                        )

                def pv(j, pt_slice):
                    for qs in range(2):
                        nc.tensor.matmul(
                            ot[:, qs * 512:(qs + 1) * 512],
                            v_s[:, j * 128:(j + 1) * 128],
                            pt_slice[:, qs * 512:(qs + 1) * 512],
                            start=(j == 0), stop=(j == NKT - 1),
                        )

                pending = []  # (kt_j, pt_slice) awaiting PV

                def flush_pv():
                    for j, ps in pending:
                        pv(j, ps)
                    pending.clear()

                for gi, (ja, jb, jc) in enumerate(GROUPS):
                    # --- A tile: k-tiles ja, jb -> [128, 2048] ---
                    sca = scap.tile([128, 2 * SB], FP32, name=f"sca_{h}_{sb}_{gi}", tag="sca")
                    qk(sca, 0, ja)
                    qk(sca, SB, jb)
                    if gi < N_DVE_A:
                        pta_i = ptap.tile(
                            [128, 2 * SB], I16, name=f"ptai_{h}_{sb}_{gi}", tag="pta"
                        )
                        nc.vector.tensor_scalar(
                            pta_i, sca, SCHRAUD_A, SCHRAUD_B,
                            mybir.AluOpType.mult, mybir.AluOpType.add,
                        )
                        pta = pta_i.bitcast(BF16)
                    else:
                        pta = ptap.tile(
                            [128, 2 * SB], BF16, name=f"pta_{h}_{sb}_{gi}", tag="pta"
                        )
                        nc.scalar.activation(
                            pta, sca, mybir.ActivationFunctionType.Exp, scale=SCALE
                        )
                    flush_pv()
                    feed("A", pta)
                    pending.append((ja, pta[:, 0:SB]))
                    pending.append((jb, pta[:, SB:2 * SB]))

                    # --- B tile: k-tile jc -> [128, 1024] ---
                    if jc is not None:
                        scb = scbp.tile([128, SB], FP32, name=f"scb_{h}_{sb}_{gi}", tag="scb")
                        qk(scb, 0, jc)
                        ptb = ptbp.tile(
                            [128, SB], BF16, name=f"ptb_{h}_{sb}_{gi}", tag="ptb"
                        )
                        nc.scalar.activation(
                            ptb, scb, mybir.ActivationFunctionType.Exp, scale=SCALE
                        )
                        flush_pv()
                        feed("B", ptb)
                        pending.append((jc, ptb))
                flush_pv()

                # Superblock drain: roots to DRAM, out psum -> sbuf -> DRAM.
                root_a = collapse("A")          # [128, 2048]
                root_af = trbp.tile([128, SB], BF16, name=f"raf_{h}_{sb}", tag="treeBc")
                nc.vector.tensor_add(root_af, root_a[:, 0:SB], root_a[:, SB:2 * SB])
                root_b = collapse("B")          # [128, 1024]
                nc.sync.dma_start(r_d[h, sb, 0], root_af)
                nc.sync.dma_start(r_d[h, sb, 1], root_b)

                ot_sb = drainp.tile([128, SB], FP32, name=f"otsb_{h}_{sb}", tag="otsb")
                nc.vector.tensor_copy(ot_sb, ot)
                nc.sync.dma_start(o_d[h, sb], ot_sb)
    nc.compile()
    return nc


def _prep_inputs(q, k, v):
    bf = ml_dtypes.bfloat16
    in_maps = []
    for c in range(N_CORES):
        hs = slice(c * HPC, (c + 1) * HPC)
        qt = np.transpose(q[:, hs, :], (1, 2, 0)).astype(bf)   # [HPC, D, S]
        kt = np.transpose(k[:, hs, :], (1, 2, 0)).astype(bf)   # [HPC, D, S]
        vh = np.transpose(v[:, hs, :], (1, 0, 2))              # [HPC, S, D]
        vp = np.ascontiguousarray(
            vh.reshape(HPC, S // 128, 128, D).transpose(0, 2, 1, 3)
        ).reshape(HPC, 128, S).astype(bf)
        in_maps.append({"qt": qt, "kt": kt, "vp": vp})
    return in_maps


def kernel(q, k, v, ring_size=None, **_unused):
    q = np.asarray(q, dtype=np.float32).reshape(S, H, D)
    k = np.asarray(k, dtype=np.float32).reshape(S, H, D)
    v = np.asarray(v, dtype=np.float32).reshape(S, H, D)

    in_maps = _prep_inputs(q, k, v)
    if "nc" not in _CACHE:
        _CACHE["nc"] = _build()
    res = run_bass_kernel_spmd(_CACHE["nc"], in_maps, list(range(N_CORES))).results

    out = np.empty((B, S, H, D), np.float32)
    for c in range(N_CORES):
        o = np.asarray(res[c]["o"])                    # [HPC, NSB, 128, SB] fp32
        r = np.asarray(res[c]["r"]).astype(np.float32)  # [HPC, NSB, 2, 128, SB]
        l = r.sum(axis=(2, 3))                          # [HPC, NSB, SB]
        on = o / l[:, :, None, :]                       # normalize per q
        # [HPC, NSB, D, SB] -> [NSB, SB, HPC, D] -> [S, HPC, D]
        out[0, :, c * HPC:(c + 1) * HPC, :] = (
            on.transpose(1, 3, 0, 2).reshape(S, HPC, D)
        )
    return out


# revision 6
# speedup vs baseline: 1.3131x; 1.3131x over previous
"""Trainium2 Bass kernel: ring attention forward == full softmax attention.

The reference's ring decomposition with the sigmoid/logsigmoid LSE merge is
mathematically exact online softmax, so the output equals plain (non-causal)
multi-head attention over the full sequence:

    out[b,q,h,:] = softmax(Q[b,q,h,:] @ K[b,:,h,:].T / sqrt(D)) @ V[b,:,h,:]

Shapes: B=1, S=4096, H=16, D=128, fp32. ring_size only affects the reference's
chunking, not the result, so it is ignored here.

Sharding: 2 heads per NeuronCore (16 heads / 8 cores), fully independent --
no cross-core communication needed (Ulysses-style head sharding).

Device algorithm per head (flash-style, transposed-scores orientation),
per 1024-wide q superblock, k-tiles j = 0..31:

      scores_T[k,q] = K_tile^T-layout @ Q^T-layout   (PE, bf16, psum fp32)
      P_T = exp(scores_T * scale)                    (ACT, FD=1024, bf16 out)
        ... or for kt in OFFLOAD: a one-instruction Schraudolph fast-exp
        on the DVE (int16 bit trick -> bf16) to offload the saturated ACT
      out_T[d,q]  += V_tile^T @ P_T                  (PE, accumulate psum)
      tree-sum of P_T k-tiles                        (DVE bf16 2x, [128,2048]
                                                      pair tiles)
  drain: out_T psum -> sbuf (DVE) -> DRAM; 2 tree roots -> DRAM raw (bf16).

Normalization (out/l) and the [d,q]->[q,d] transpose happen on the host
during unsharding: l = per-q partition-sum of the two roots. This removes
all PE transposes and the psum-resident l machinery, freeing 2 PSUM banks
which triple-buffer the score tiles (PSUM: 3x2 scores + 2 out = 8 banks),
so ACT never stalls on QK and PE runs 2 tiles ahead (PV at lag 2).
"""

import numpy as np
import ml_dtypes
from contextlib import ExitStack

import concourse.bass as bass
import concourse.bacc as bacc
import concourse.mybir as mybir
import concourse.tile as tile
from concourse.bass_utils import run_bass_kernel_spmd

B, S, H, D = 1, 4096, 16, 128
N_CORES = 8
HPC = H // N_CORES          # heads per core
SB = 1024                   # q superblock width
NSB = S // SB
NKT = S // 128              # 32 k-tiles of 128 keys
SCALE = float(1.0 / np.sqrt(D))
BF16 = mybir.dt.bfloat16
FP32 = mybir.dt.float32
I16 = mybir.dt.int16

# k-tiles whose exp runs as a Schraudolph fast-exp on the DVE instead of
# exact exp on the saturated ACT engine. bf16(exp(x*SCALE)) ~=
# bitcast_bf16(int16(x*A + B)); end-to-end rel_rms ~1e-3 per offloaded
# tile pair (host-validated: 6 tiles -> ~6e-3 including bf16 matmul noise).
OFFLOAD = ()
SCHRAUD_SIGMA = 0.06
SCHRAUD_A = float(SCALE * 128.0 * np.log2(np.e))
SCHRAUD_B = float(128.0 * (127.0 - SCHRAUD_SIGMA))

# Matmul free-dim chunk (512 = one psum bank; probe 1024 = two banks).
MM_FD = 512

_CACHE = {}


def _build():
    nc = bacc.Bacc("TRN2", target_bir_lowering=False, debug=False)
    # Inputs per core (host pre-arranged, bf16):
    #   qt/kt: [head, d, s]  (transposed layout, d on partitions)
    #   vp:    [head, p, t*128+c] where vp[h, p, 128t+c] = V[128t+p, c]
    qt_d = nc.dram_tensor("qt", [HPC, 128, S], BF16, kind="ExternalInput")
    kt_d = nc.dram_tensor("kt", [HPC, 128, S], BF16, kind="ExternalInput")
    vp_d = nc.dram_tensor("vp", [HPC, 128, S], BF16, kind="ExternalInput")
    # Outputs: unnormalized out in [d, q] layout + raw tree roots (host
    # finishes l = partition-sum of roots, then out/l and transpose).
    o_d = nc.dram_tensor("o", [HPC, NSB, 128, SB], FP32, kind="ExternalOutput")
    r_d = nc.dram_tensor("r", [HPC, NSB, 128, SB], BF16, kind="ExternalOutput")

    with ExitStack() as ctx:
        tc = ctx.enter_context(tile.TileContext(nc))

        qkv = ctx.enter_context(tc.tile_pool(name="qkv", bufs=2))
        ptp = ctx.enter_context(tc.tile_pool(name="ptp", bufs=3))
        trp = ctx.enter_context(tc.tile_pool(name="trp", bufs=2))
        drainp = ctx.enter_context(tc.tile_pool(name="drainp", bufs=2))

        # PSUM budget: 8 banks of [128, 512 fp32].
        scp = ctx.enter_context(tc.tile_pool(name="scp", bufs=3, space="PSUM"))  # 6 banks
        otp = ctx.enter_context(tc.tile_pool(name="otp", bufs=1, space="PSUM"))  # 2 banks

        for h in range(HPC):
            # Chunked loads so the first QK can start before full tensors land.
            qt_s = qkv.tile([128, S], BF16, name=f"qt{h}", tag="qt")
            kt_s = qkv.tile([128, S], BF16, name=f"kt{h}", tag="kt")
            v_s = qkv.tile([128, S], BF16, name=f"v{h}", tag="v")
            for ch in range(4):
                cs = slice(ch * (S // 4), (ch + 1) * (S // 4))
                nc.sync.dma_start(kt_s[:, cs], kt_d[h][:, cs])
                nc.sync.dma_start(qt_s[:, cs], qt_d[h][:, cs])
                nc.sync.dma_start(v_s[:, cs], vp_d[h][:, cs])

            for sb in range(NSB):
                q0 = sb * SB
                ot = otp.tile([128, SB], FP32, name=f"ot_{h}_{sb}", tag="ot")

                # Binary tree-sum of P_T pair tiles on the DVE (bf16 2x
                # mode, [128, 2048] ops). Two roots go to DRAM raw; the
                # host finishes the 128-partition sum for l.
                tree = {}
                treectr = [0]

                def feed(t, level=0):
                    while level in tree:
                        prev = tree.pop(level)
                        treectr[0] += 1
                        nt = trp.tile(
                            [128, 2 * SB], BF16,
                            name=f"tr_{h}_{sb}_{level}_{treectr[0]}",
                            tag=f"tree{level}",
                        )
                        nc.vector.tensor_add(nt, prev, t)
                        t = nt
                        level += 1
                    tree[level] = t

                def qk(sc_dst, j):
                    for qs in range(SB // MM_FD):
                        nc.tensor.matmul(
                            sc_dst[:, qs * MM_FD:(qs + 1) * MM_FD],
                            kt_s[:, j * 128:(j + 1) * 128],
                            qt_s[:, q0 + qs * MM_FD: q0 + (qs + 1) * MM_FD],
                            start=True, stop=True,
                        )

                def pv(j, pt_slice):
                    for qs in range(SB // MM_FD):
                        nc.tensor.matmul(
                            ot[:, qs * MM_FD:(qs + 1) * MM_FD],
                            v_s[:, j * 128:(j + 1) * 128],
                            pt_slice[:, qs * MM_FD:(qs + 1) * MM_FD],
                            start=(j == 0), stop=(j == NKT - 1),
                        )

                # pt pair tiles: [128, 2048] bf16, halves written by exp of
                # k-tiles 2p and 2p+1 (subtile deps let PV consume halves).
                pt_pairs = {}
                pv_queue = []  # (kt_j, pt_slice) in kt order, flushed at lag 2

                for j in range(NKT):
                    sc = scp.tile([128, SB], FP32, name=f"sc_{h}_{sb}_{j}", tag="sc")
                    qk(sc, j)
                    p = j // 2
                    if p not in pt_pairs:
                        pt_pairs[p] = ptp.tile(
                            [128, 2 * SB], BF16, name=f"pt_{h}_{sb}_{p}", tag="pt"
                        )
                    half = pt_pairs[p][:, (j % 2) * SB:(j % 2 + 1) * SB]
                    if j in OFFLOAD:
                        nc.vector.tensor_scalar(
                            half.bitcast(I16), sc, SCHRAUD_A, SCHRAUD_B,
                            mybir.AluOpType.mult, mybir.AluOpType.add,
                        )
                    else:
                        nc.scalar.activation(
                            half, sc, mybir.ActivationFunctionType.Exp, scale=SCALE
                        )
                    if j % 2 == 1:
                        feed(pt_pairs.pop(p))
                    pv_queue.append((j, half))
                    if len(pv_queue) > 2:
                        pv(*pv_queue.pop(0))
                    if j == NKT - 1:
                        for item in pv_queue:
                            pv(*item)
                        pv_queue.clear()

                # Superblock drain: out psum -> sbuf -> DRAM first (frees ot
                # for the next superblock), then the two tree roots.
                ot_sb = drainp.tile([128, SB], FP32, name=f"otsb_{h}_{sb}", tag="otsb")
                nc.vector.tensor_copy(ot_sb, ot)
                nc.sync.dma_start(o_d[h, sb], ot_sb)

                levels = sorted(tree)
                t = tree.pop(levels[0])
                for lv in levels[1:]:
                    prev = tree.pop(lv)
                    treectr[0] += 1
                    nt = trp.tile(
                        [128, 2 * SB], BF16, name=f"trc_{h}_{sb}_{treectr[0]}",
                        tag="treec",
                    )
                    nc.vector.tensor_add(nt, prev, t)
                    t = nt
                root_f = drainp.tile([128, SB], BF16, name=f"rf_{h}_{sb}", tag="rf")
                nc.vector.tensor_add(root_f, t[:, 0:SB], t[:, SB:2 * SB])
                nc.sync.dma_start(r_d[h, sb], root_f)
    nc.compile()
    return nc


def _prep_inputs(q, k, v):
    bf = ml_dtypes.bfloat16
    in_maps = []
    for c in range(N_CORES):
        hs = slice(c * HPC, (c + 1) * HPC)
        qt = np.transpose(q[:, hs, :], (1, 2, 0)).astype(bf)   # [HPC, D, S]
        kt = np.transpose(k[:, hs, :], (1, 2, 0)).astype(bf)   # [HPC, D, S]
        vh = np.transpose(v[:, hs, :], (1, 0, 2))              # [HPC, S, D]
        vp = np.ascontiguousarray(
            vh.reshape(HPC, S // 128, 128, D).transpose(0, 2, 1, 3)
        ).reshape(HPC, 128, S).astype(bf)
        in_maps.append({"qt": qt, "kt": kt, "vp": vp})
    return in_maps


def kernel(q, k, v, ring_size=None, **_unused):
    q = np.asarray(q, dtype=np.float32).reshape(S, H, D)
    k = np.asarray(k, dtype=np.float32).reshape(S, H, D)
    v = np.asarray(v, dtype=np.float32).reshape(S, H, D)

    in_maps = _prep_inputs(q, k, v)
    if "nc" not in _CACHE:
        _CACHE["nc"] = _build()
    res = run_bass_kernel_spmd(_CACHE["nc"], in_maps, list(range(N_CORES))).results

    out = np.empty((B, S, H, D), np.float32)
    for c in range(N_CORES):
        o = np.asarray(res[c]["o"])                     # [HPC, NSB, 128, SB] fp32
        r = np.asarray(res[c]["r"]).astype(np.float32)  # [HPC, NSB, 128, SB]
        l = r.sum(axis=2)                               # [HPC, NSB, SB]
        on = o / l[:, :, None, :]                       # normalize per q
        # [HPC, NSB, D, SB] -> [NSB, SB, HPC, D] -> [S, HPC, D]
        out[0, :, c * HPC:(c + 1) * HPC, :] = (
            on.transpose(1, 3, 0, 2).reshape(S, HPC, D)
        )
    return out


# revision 11
# speedup vs baseline: 1.3512x; 1.0290x over previous
"""Trainium2 Bass kernel: ring attention forward == full softmax attention.

The reference's ring decomposition with the sigmoid/logsigmoid LSE merge is
mathematically exact online softmax, so the output equals plain (non-causal)
multi-head attention over the full sequence:

    out[b,q,h,:] = softmax(Q[b,q,h,:] @ K[b,:,h,:].T / sqrt(D)) @ V[b,:,h,:]

Shapes: B=1, S=4096, H=16, D=128, fp32. ring_size only affects the reference's
chunking, not the result, so it is ignored here.

Sharding: 2 heads per NeuronCore (16 heads / 8 cores), fully independent --
no cross-core communication needed (Ulysses-style head sharding).

Device algorithm per head (flash-style, transposed-scores orientation),
per 1024-wide q superblock, k-tiles j = 0..31:

      scores_T[k,q] = K_tile^T-layout @ Q^T-layout   (PE, bf16, psum fp32)
      P_T = exp(scores_T * scale)                    (ACT, FD=1024, bf16 out)
        ... or for kt in OFFLOAD: a one-instruction Schraudolph fast-exp
        on the DVE (int16 bit trick -> bf16) to offload the saturated ACT
      out_T[d,q]  += V_tile^T @ P_T                  (PE, accumulate psum)
      tree-sum of P_T k-tiles                        (DVE bf16 2x, [128,2048]
                                                      pair tiles)
  drain: out_T psum -> sbuf (DVE) -> DRAM; 2 tree roots -> DRAM raw (bf16).

Normalization (out/l) and the [d,q]->[q,d] transpose happen on the host
during unsharding: l = per-q partition-sum of the two roots. This removes
all PE transposes and the psum-resident l machinery, freeing 2 PSUM banks
which triple-buffer the score tiles (PSUM: 3x2 scores + 2 out = 8 banks),
so ACT never stalls on QK and PE runs 2 tiles ahead (PV at lag 2).
"""

import numpy as np
import ml_dtypes
from contextlib import ExitStack

import concourse.bass as bass
import concourse.bacc as bacc
import concourse.mybir as mybir
import concourse.tile as tile
from concourse.bass_utils import run_bass_kernel_spmd

B, S, H, D = 1, 4096, 16, 128
N_CORES = 8
HPC = H // N_CORES          # heads per core
SB = 1024                   # q superblock width
NSB = S // SB
NKT = S // 128              # 32 k-tiles of 128 keys
SCALE = float(1.0 / np.sqrt(D))
BF16 = mybir.dt.bfloat16
FP32 = mybir.dt.float32
I16 = mybir.dt.int16

# k-tiles whose exp runs as a Schraudolph fast-exp on the DVE instead of
# exact exp on the saturated ACT engine. bf16(exp(x*SCALE)) ~=
# bitcast_bf16(int16(x*A + B)); end-to-end rel_rms ~1e-3 per offloaded
# tile pair (host-validated: 6 tiles -> ~6e-3 including bf16 matmul noise).
OFFLOAD = (2, 7, 12, 17, 22, 27)
SCHRAUD_SIGMA = 0.06
SCHRAUD_A = float(SCALE * 128.0 * np.log2(np.e))
SCHRAUD_B = float(128.0 * (127.0 - SCHRAUD_SIGMA))

# Tree level at which partial sums stop on-device and go to the host
# (level 2 tiles each cover 8 k-tiles; 4 roots per superblock).
ROOT_LEVEL = 2
N_ROOTS = NKT // 2 // (1 << ROOT_LEVEL)

# Matmul free-dim chunk (512 = one psum bank; probe 1024 = two banks).
MM_FD = 512

_CACHE = {}


def _build():
    nc = bacc.Bacc("TRN2", target_bir_lowering=False, debug=False)
    # Inputs per core (host pre-arranged, bf16):
    #   qt/kt: [head, d, s]  (transposed layout, d on partitions)
    #   vp:    [head, p, t*128+c] where vp[h, p, 128t+c] = V[128t+p, c]
    qt_d = nc.dram_tensor("qt", [HPC, 128, S], BF16, kind="ExternalInput")
    kt_d = nc.dram_tensor("kt", [HPC, 128, S], BF16, kind="ExternalInput")
    vp_d = nc.dram_tensor("vp", [HPC, 128, S], BF16, kind="ExternalInput")
    # Outputs: unnormalized out in [d, q] layout + raw tree roots (host
    # finishes l = partition-sum of roots, then out/l and transpose).
    o_d = nc.dram_tensor("o", [HPC, NSB, 128, SB], FP32, kind="ExternalOutput")
    r_d = nc.dram_tensor(
        "r", [HPC, NSB, N_ROOTS, 128, 2 * SB], BF16, kind="ExternalOutput"
    )

    with ExitStack() as ctx:
        tc = ctx.enter_context(tile.TileContext(nc))

        qkv = ctx.enter_context(tc.tile_pool(name="qkv", bufs=2))
        ptp = ctx.enter_context(tc.tile_pool(name="ptp", bufs=3))
        trp = ctx.enter_context(tc.tile_pool(name="trp", bufs=2))
        drainp = ctx.enter_context(tc.tile_pool(name="drainp", bufs=2))

        # PSUM budget: 8 banks of [128, 512 fp32].
        scp = ctx.enter_context(tc.tile_pool(name="scp", bufs=3, space="PSUM"))  # 6 banks
        otp = ctx.enter_context(tc.tile_pool(name="otp", bufs=1, space="PSUM"))  # 2 banks

        for h in range(HPC):
            # Chunked loads so the first QK can start before full tensors land.
            qt_s = qkv.tile([128, S], BF16, name=f"qt{h}", tag="qt")
            kt_s = qkv.tile([128, S], BF16, name=f"kt{h}", tag="kt")
            v_s = qkv.tile([128, S], BF16, name=f"v{h}", tag="v")
            for ch in range(4):
                cs = slice(ch * (S // 4), (ch + 1) * (S // 4))
                nc.sync.dma_start(kt_s[:, cs], kt_d[h][:, cs])
                nc.sync.dma_start(qt_s[:, cs], qt_d[h][:, cs])
                nc.sync.dma_start(v_s[:, cs], vp_d[h][:, cs])

            for sb in range(NSB):
                q0 = sb * SB
                ot = otp.tile([128, SB], FP32, name=f"ot_{h}_{sb}", tag="ot")

                # Binary tree-sum of P_T pair tiles on the DVE (bf16 2x
                # mode, [128, 2048] ops), stopped at ROOT_LEVEL; the roots
                # stream to DRAM and the host finishes the reduction for l.
                tree = {}
                treectr = [0]
                rootctr = [0]

                def feed(t, level=0):
                    while level in tree:
                        prev = tree.pop(level)
                        treectr[0] += 1
                        nt = trp.tile(
                            [128, 2 * SB], BF16,
                            name=f"tr_{h}_{sb}_{level}_{treectr[0]}",
                            tag=f"tree{level}",
                        )
                        nc.vector.tensor_add(nt, prev, t)
                        t = nt
                        level += 1
                        if level == ROOT_LEVEL:
                            nc.sync.dma_start(r_d[h, sb, rootctr[0]], t)
                            rootctr[0] += 1
                            return
                    tree[level] = t

                def qk(sc_dst, j):
                    for qs in range(SB // MM_FD):
                        nc.tensor.matmul(
                            sc_dst[:, qs * MM_FD:(qs + 1) * MM_FD],
                            kt_s[:, j * 128:(j + 1) * 128],
                            qt_s[:, q0 + qs * MM_FD: q0 + (qs + 1) * MM_FD],
                            start=True, stop=True,
                        )

                def pv(j, pt_slice):
                    for qs in range(SB // MM_FD):
                        nc.tensor.matmul(
                            ot[:, qs * MM_FD:(qs + 1) * MM_FD],
                            v_s[:, j * 128:(j + 1) * 128],
                            pt_slice[:, qs * MM_FD:(qs + 1) * MM_FD],
                            start=(j == 0), stop=(j == NKT - 1),
                        )

                # pt pair tiles: [128, 2048] bf16, halves written by exp of
                # k-tiles 2p and 2p+1 (subtile deps let PV consume halves).
                pt_pairs = {}
                pv_queue = []  # (kt_j, pt_slice) in kt order, flushed at lag 2

                for j in range(NKT):
                    sc = scp.tile([128, SB], FP32, name=f"sc_{h}_{sb}_{j}", tag="sc")
                    qk(sc, j)
                    p = j // 2
                    if p not in pt_pairs:
                        pt_pairs[p] = ptp.tile(
                            [128, 2 * SB], BF16, name=f"pt_{h}_{sb}_{p}", tag="pt"
                        )
                    half = pt_pairs[p][:, (j % 2) * SB:(j % 2 + 1) * SB]
                    if j in OFFLOAD:
                        nc.vector.tensor_scalar(
                            half.bitcast(I16), sc, SCHRAUD_A, SCHRAUD_B,
                            mybir.AluOpType.mult, mybir.AluOpType.add,
                        )
                    else:
                        nc.scalar.activation(
                            half, sc, mybir.ActivationFunctionType.Exp, scale=SCALE
                        )
                    if j % 2 == 1:
                        feed(pt_pairs.pop(p))
                    pv_queue.append((j, half))
                    if len(pv_queue) > 2:
                        pv(*pv_queue.pop(0))
                    if j == NKT - 1:
                        for item in pv_queue:
                            pv(*item)
                        pv_queue.clear()

                assert not tree and rootctr[0] == N_ROOTS

                # Superblock drain: out psum -> sbuf -> DRAM (frees ot for
                # the next superblock).
                ot_sb = drainp.tile([128, SB], FP32, name=f"otsb_{h}_{sb}", tag="otsb")
                nc.vector.tensor_copy(ot_sb, ot)
                nc.sync.dma_start(o_d[h, sb], ot_sb)
    nc.compile()
    return nc


def _prep_inputs(q, k, v):
    bf = ml_dtypes.bfloat16
    in_maps = []
    for c in range(N_CORES):
        hs = slice(c * HPC, (c + 1) * HPC)
        qt = np.transpose(q[:, hs, :], (1, 2, 0)).astype(bf)   # [HPC, D, S]
        kt = np.transpose(k[:, hs, :], (1, 2, 0)).astype(bf)   # [HPC, D, S]
        vh = np.transpose(v[:, hs, :], (1, 0, 2))              # [HPC, S, D]
        vp = np.ascontiguousarray(
            vh.reshape(HPC, S // 128, 128, D).transpose(0, 2, 1, 3)
        ).reshape(HPC, 128, S).astype(bf)
        in_maps.append({"qt": qt, "kt": kt, "vp": vp})
    return in_maps


def kernel(q, k, v, ring_size=None, **_unused):
    q = np.asarray(q, dtype=np.float32).reshape(S, H, D)
    k = np.asarray(k, dtype=np.float32).reshape(S, H, D)
    v = np.asarray(v, dtype=np.float32).reshape(S, H, D)

    in_maps = _prep_inputs(q, k, v)
    if "nc" not in _CACHE:
        _CACHE["nc"] = _build()
    res = run_bass_kernel_spmd(_CACHE["nc"], in_maps, list(range(N_CORES))).results

    out = np.empty((B, S, H, D), np.float32)
    for c in range(N_CORES):
        o = np.asarray(res[c]["o"])                     # [HPC, NSB, 128, SB] fp32
        r = np.asarray(res[c]["r"]).astype(np.float32)  # [HPC,NSB,N_ROOTS,128,2SB]
        # roots are [128, 2*SB]: two SB-wide halves (k-tile pair layout)
        l = r.sum(axis=(2, 3)).reshape(HPC, NSB, 2, SB).sum(axis=2)  # [HPC,NSB,SB]
        on = o / l[:, :, None, :]                       # normalize per q
        # [HPC, NSB, D, SB] -> [NSB, SB, HPC, D] -> [S, HPC, D]
        out[0, :, c * HPC:(c + 1) * HPC, :] = (
            on.transpose(1, 3, 0, 2).reshape(S, HPC, D)
        )
    return out


# revision 16
# speedup vs baseline: 1.3548x; 1.0027x over previous
"""Trainium2 Bass kernel: ring attention forward == full softmax attention.

The reference's ring decomposition with the sigmoid/logsigmoid LSE merge is
mathematically exact online softmax, so the output equals plain (non-causal)
multi-head attention over the full sequence:

    out[b,q,h,:] = softmax(Q[b,q,h,:] @ K[b,:,h,:].T / sqrt(D)) @ V[b,:,h,:]

Shapes: B=1, S=4096, H=16, D=128, fp32. ring_size only affects the reference's
chunking, not the result, so it is ignored here.

Sharding: 2 heads per NeuronCore (16 heads / 8 cores), fully independent --
no cross-core communication needed (Ulysses-style head sharding).

Device algorithm per head (flash-style, transposed-scores orientation),
per 1024-wide q superblock, k-tiles j = 0..31:

      scores_T[k,q] = K_tile^T-layout @ Q^T-layout   (PE, bf16, psum fp32)
      P_T = exp(scores_T * scale)                    (ACT, FD=1024, bf16 out)
        ... or for kt in OFFLOAD: a one-instruction Schraudolph fast-exp
        on the DVE (int16 bit trick -> bf16) to offload the saturated ACT
      out_T[d,q]  += V_tile^T @ P_T                  (PE, accumulate psum)
      tree-sum of P_T k-tiles                        (DVE bf16 2x, [128,2048]
                                                      pair tiles)
  drain: out_T psum -> sbuf (DVE) -> DRAM; 2 tree roots -> DRAM raw (bf16).

Normalization (out/l) and the [d,q]->[q,d] transpose happen on the host
during unsharding: l = per-q partition-sum of the two roots. This removes
all PE transposes and the psum-resident l machinery, freeing 2 PSUM banks
which triple-buffer the score tiles (PSUM: 3x2 scores + 2 out = 8 banks),
so ACT never stalls on QK and PE runs 2 tiles ahead (PV at lag 2).
"""

import numpy as np
import ml_dtypes
from contextlib import ExitStack

import concourse.bass as bass
import concourse.bacc as bacc
import concourse.mybir as mybir
import concourse.tile as tile
from concourse.bass_utils import run_bass_kernel_spmd

B, S, H, D = 1, 4096, 16, 128
N_CORES = 8
HPC = H // N_CORES          # heads per core
SB = 1024                   # q superblock width
NSB = S // SB
NKT = S // 128              # 32 k-tiles of 128 keys
SCALE = float(1.0 / np.sqrt(D))
BF16 = mybir.dt.bfloat16
FP32 = mybir.dt.float32
I16 = mybir.dt.int16

# k-tiles whose exp runs as a Schraudolph fast-exp on the DVE instead of
# exact exp on the saturated ACT engine. bf16(exp(x*SCALE)) ~=
# bitcast_bf16(int16(x*A + B)); end-to-end rel_rms ~1e-3 per offloaded
# tile pair (host-validated: 6 tiles -> ~6e-3 including bf16 matmul noise).
OFFLOAD = (2, 9, 17, 25)
SCHRAUD_SIGMA = 0.06
SCHRAUD_A = float(SCALE * 128.0 * np.log2(np.e))
SCHRAUD_B = float(128.0 * (127.0 - SCHRAUD_SIGMA))

# Tree level at which partial sums stop on-device and go to the host
# (level 2 tiles each cover 8 k-tiles; 4 roots per superblock). The final
# superblock flushes its last two pair tiles and their level-1 sibling
# directly (slots 3..5) to shorten the pipeline tail.
ROOT_LEVEL = 2
N_ROOT_SLOTS = NKT // 2 // (1 << ROOT_LEVEL) + 2

# Matmul free-dim chunks (512 = one psum bank; walrus rejects 2-bank MMs).
QK_FD = 512
PV_FD = 512

_CACHE = {}


def _build():
    nc = bacc.Bacc("TRN2", target_bir_lowering=False, debug=False)
    # Inputs per core (host pre-arranged, bf16):
    #   qt/kt: [head, d, s]  (transposed layout, d on partitions)
    #   vp:    [head, p, t*128+c] where vp[h, p, 128t+c] = V[128t+p, c]
    qt_d = nc.dram_tensor("qt", [HPC, 128, S], BF16, kind="ExternalInput")
    kt_d = nc.dram_tensor("kt", [HPC, 128, S], BF16, kind="ExternalInput")
    vp_d = nc.dram_tensor("vp", [HPC, 128, S], BF16, kind="ExternalInput")
    # Outputs: unnormalized out in [d, q] layout + raw tree roots (host
    # finishes l = partition-sum of roots, then out/l and transpose).
    o_d = nc.dram_tensor("o", [HPC, NSB, 128, SB], FP32, kind="ExternalOutput")
    r_d = nc.dram_tensor(
        "r", [HPC, NSB, N_ROOT_SLOTS, 128, 2 * SB], BF16, kind="ExternalOutput"
    )

    with ExitStack() as ctx:
        tc = ctx.enter_context(tile.TileContext(nc))

        qkv = ctx.enter_context(tc.tile_pool(name="qkv", bufs=2))
        ptp = ctx.enter_context(tc.tile_pool(name="ptp", bufs=3))
        trp = ctx.enter_context(tc.tile_pool(name="trp", bufs=2))
        drainp = ctx.enter_context(tc.tile_pool(name="drainp", bufs=2))

        # PSUM budget: 8 banks of [128, 512 fp32].
        scp = ctx.enter_context(tc.tile_pool(name="scp", bufs=3, space="PSUM"))  # 6 banks
        otp = ctx.enter_context(tc.tile_pool(name="otp", bufs=1, space="PSUM"))  # 2 banks

        # HAM warm-up: ~26 dummy matmuls during the initial DMA wait so the
        # PE clock gate is at 8/8 (2.4 GHz) when the real QK stream starts.
        dummy = qkv.tile([128, 512], BF16, name="dummy", tag="dummy", bufs=1)
        nc.gpsimd.memset(dummy, 0.0)
        warm = scp.tile([128, SB], FP32, name="warm", tag="sc")
        for i in range(10):
            nc.tensor.matmul(
                warm[:, (i % 2) * 512:(i % 2 + 1) * 512],
                dummy[:, 0:128], dummy, start=True, stop=True,
            )

        for h in range(HPC):
            # Chunked loads so the first QK can start before full tensors land.
            qt_s = qkv.tile([128, S], BF16, name=f"qt{h}", tag="qt")
            kt_s = qkv.tile([128, S], BF16, name=f"kt{h}", tag="kt")
            v_s = qkv.tile([128, S], BF16, name=f"v{h}", tag="v")
            bounds = [0, 512, 1024, 2048, 3072, 4096]
            for ch in range(len(bounds) - 1):
                cs = slice(bounds[ch], bounds[ch + 1])
                nc.sync.dma_start(kt_s[:, cs], kt_d[h][:, cs])
                nc.sync.dma_start(qt_s[:, cs], qt_d[h][:, cs])
                nc.sync.dma_start(v_s[:, cs], vp_d[h][:, cs])

            for sb in range(NSB):
                q0 = sb * SB
                last_unit = (h == HPC - 1) and (sb == NSB - 1)
                ot = otp.tile([128, SB], FP32, name=f"ot_{h}_{sb}", tag="ot")

                # Binary tree-sum of P_T pair tiles on the DVE (bf16 2x
                # mode, [128, 2048] ops), stopped at ROOT_LEVEL; the roots
                # stream to DRAM and the host finishes the reduction for l.
                tree = {}
                treectr = [0]
                rootctr = [0]

                def feed(t, level=0, direct=False):
                    if direct:
                        # tail-trim: ship this tile as a root without merging
                        nc.sync.dma_start(r_d[h, sb, rootctr[0]], t)
                        rootctr[0] += 1
                        return
                    while level in tree:
                        prev = tree.pop(level)
                        treectr[0] += 1
                        nt = trp.tile(
                            [128, 2 * SB], BF16,
                            name=f"tr_{h}_{sb}_{level}_{treectr[0]}",
                            tag=f"tree{level}",
                        )
                        nc.vector.tensor_add(nt, prev, t)
                        t = nt
                        level += 1
                        if level == ROOT_LEVEL:
                            nc.sync.dma_start(r_d[h, sb, rootctr[0]], t)
                            rootctr[0] += 1
                            return
                    tree[level] = t

                def qk(sc_dst, j):
                    for qs in range(SB // QK_FD):
                        nc.tensor.matmul(
                            sc_dst[:, qs * QK_FD:(qs + 1) * QK_FD],
                            kt_s[:, j * 128:(j + 1) * 128],
                            qt_s[:, q0 + qs * QK_FD: q0 + (qs + 1) * QK_FD],
                            start=True, stop=True,
                        )

                def pv(j, pt_slice):
                    for qs in range(SB // PV_FD):
                        nc.tensor.matmul(
                            ot[:, qs * PV_FD:(qs + 1) * PV_FD],
                            v_s[:, j * 128:(j + 1) * 128],
                            pt_slice[:, qs * PV_FD:(qs + 1) * PV_FD],
                            start=(j == 0), stop=(j == NKT - 1),
                        )

                # pt pair tiles: [128, 2048] bf16, halves written by exp of
                # k-tiles 2p and 2p+1 (subtile deps let PV consume halves).
                pt_pairs = {}
                pv_queue = []  # (kt_j, pt_slice) in kt order, flushed at lag 2

                for j in range(NKT):
                    sc = scp.tile([128, SB], FP32, name=f"sc_{h}_{sb}_{j}", tag="sc")
                    qk(sc, j)
                    p = j // 2
                    if p not in pt_pairs:
                        pt_pairs[p] = ptp.tile(
                            [128, 2 * SB], BF16, name=f"pt_{h}_{sb}_{p}", tag="pt"
                        )
                    half = pt_pairs[p][:, (j % 2) * SB:(j % 2 + 1) * SB]
                    if j in OFFLOAD:
                        nc.vector.tensor_scalar(
                            half.bitcast(I16), sc, SCHRAUD_A, SCHRAUD_B,
                            mybir.AluOpType.mult, mybir.AluOpType.add,
                        )
                    else:
                        nc.scalar.activation(
                            half, sc, mybir.ActivationFunctionType.Exp, scale=SCALE
                        )
                    if j % 2 == 1:
                        if last_unit and j >= NKT - 4:
                            # flush the final two pair tiles raw; their
                            # level-1 sibling flushes below
                            feed(pt_pairs.pop(p), direct=True)
                        else:
                            feed(pt_pairs.pop(p))
                    pv_queue.append((j, half))
                    if len(pv_queue) > 2:
                        pv(*pv_queue.pop(0))
                    if j == NKT - 1:
                        for item in pv_queue:
                            pv(*item)
                        pv_queue.clear()

                if last_unit:
                    # ship the leftover level-1 tile (pairs 12+13) raw
                    (lv,) = sorted(tree)
                    feed(tree.pop(lv), direct=True)
                assert not tree

                # Superblock drain: out psum -> sbuf -> DRAM (frees ot for
                # the next superblock). Copy runs on the scalar engine,
                # which has slack, keeping the DVE queue clear at the
                # superblock boundary.
                ot_sb = drainp.tile([128, SB], FP32, name=f"otsb_{h}_{sb}", tag="otsb")
                nc.scalar.copy(ot_sb, ot)
                nc.sync.dma_start(o_d[h, sb], ot_sb)
    nc.compile()
    return nc


def _prep_inputs(q, k, v):
    bf = ml_dtypes.bfloat16
    in_maps = []
    for c in range(N_CORES):
        hs = slice(c * HPC, (c + 1) * HPC)
        qt = np.transpose(q[:, hs, :], (1, 2, 0)).astype(bf)   # [HPC, D, S]
        kt = np.transpose(k[:, hs, :], (1, 2, 0)).astype(bf)   # [HPC, D, S]
        vh = np.transpose(v[:, hs, :], (1, 0, 2))              # [HPC, S, D]
        vp = np.ascontiguousarray(
            vh.reshape(HPC, S // 128, 128, D).transpose(0, 2, 1, 3)
        ).reshape(HPC, 128, S).astype(bf)
        in_maps.append({"qt": qt, "kt": kt, "vp": vp})
    return in_maps


def kernel(q, k, v, ring_size=None, **_unused):
    q = np.asarray(q, dtype=np.float32).reshape(S, H, D)
    k = np.asarray(k, dtype=np.float32).reshape(S, H, D)
    v = np.asarray(v, dtype=np.float32).reshape(S, H, D)

    in_maps = _prep_inputs(q, k, v)
    if "nc" not in _CACHE:
        _CACHE["nc"] = _build()
    res = run_bass_kernel_spmd(_CACHE["nc"], in_maps, list(range(N_CORES))).results

    out = np.empty((B, S, H, D), np.float32)
    for c in range(N_CORES):
        o = np.asarray(res[c]["o"])                     # [HPC, NSB, 128, SB] fp32
        r = np.asarray(res[c]["r"]).astype(np.float32)  # [HPC,NSB,SLOTS,128,2SB]
        # normal superblocks fill root slots 0..3; the final one fills 0..5
        # (3 level-2 roots + 2 raw pairs + 1 level-1). Zero out unwritten
        # slots before summing.
        r[:HPC - 1, :, 4:] = 0.0
        r[HPC - 1, :NSB - 1, 4:] = 0.0
        # roots are [128, 2*SB]: two SB-wide halves (k-tile pair layout)
        l = r.sum(axis=(2, 3)).reshape(HPC, NSB, 2, SB).sum(axis=2)  # [HPC,NSB,SB]
        on = o / l[:, :, None, :]                       # normalize per q
        # [HPC, NSB, D, SB] -> [NSB, SB, HPC, D] -> [S, HPC, D]
        out[0, :, c * HPC:(c + 1) * HPC, :] = (
            on.transpose(1, 3, 0, 2).reshape(S, HPC, D)
        )
    return out


# revision 17
# speedup vs baseline: 1.4340x; 1.0585x over previous
"""Trainium2 Bass kernel: ring attention forward == full softmax attention.

The reference's ring decomposition with the sigmoid/logsigmoid LSE merge is
mathematically exact online softmax, so the output equals plain (non-causal)
multi-head attention over the full sequence:

    out[b,q,h,:] = softmax(Q[b,q,h,:] @ K[b,:,h,:].T / sqrt(D)) @ V[b,:,h,:]

Shapes: B=1, S=4096, H=16, D=128, fp32. ring_size only affects the reference's
chunking, not the result, so it is ignored here.

Sharding: 2 heads per NeuronCore (16 heads / 8 cores), fully independent --
no cross-core communication needed (Ulysses-style head sharding).

Device algorithm per head (flash-style, transposed-scores orientation),
per 1024-wide q superblock, k-tiles j = 0..31:

      scores_T[k,q] = K_tile^T-layout @ Q^T-layout   (PE, bf16, psum fp32)
      P_T = exp(scores_T * scale)                    (ACT, FD=1024, bf16 out)
        ... or for kt in OFFLOAD: a one-instruction Schraudolph fast-exp
        on the DVE (int16 bit trick -> bf16) to offload the saturated ACT
      out_T[d,q]  += V_tile^T @ P_T                  (PE, accumulate psum)
      tree-sum of P_T k-tiles                        (DVE bf16 2x, [128,2048]
                                                      pair tiles)
  drain: out_T psum -> sbuf (DVE) -> DRAM; 2 tree roots -> DRAM raw (bf16).

Normalization (out/l) and the [d,q]->[q,d] transpose happen on the host
during unsharding: l = per-q partition-sum of the two roots. This removes
all PE transposes and the psum-resident l machinery, freeing 2 PSUM banks
which triple-buffer the score tiles (PSUM: 3x2 scores + 2 out = 8 banks),
so ACT never stalls on QK and PE runs 2 tiles ahead (PV at lag 2).
"""

import numpy as np
import ml_dtypes
from contextlib import ExitStack

import concourse.bass as bass
import concourse.bacc as bacc
import concourse.mybir as mybir
import concourse.tile as tile
from concourse.bass_utils import run_bass_kernel_spmd

B, S, H, D = 1, 4096, 16, 128
N_CORES = 8
HPC = H // N_CORES          # heads per core
SB = 1024                   # q superblock width
NSB = S // SB
NKT = S // 128              # 32 k-tiles of 128 keys
SCALE = float(1.0 / np.sqrt(D))
BF16 = mybir.dt.bfloat16
FP32 = mybir.dt.float32
I16 = mybir.dt.int16

# k-tiles whose exp runs as a Schraudolph fast-exp on the DVE instead of
# exact exp on the saturated ACT engine. bf16(exp(x*SCALE)) ~=
# bitcast_bf16(int16(x*A + B)); end-to-end rel_rms ~1e-3 per offloaded
# tile pair (host-validated: 6 tiles -> ~6e-3 including bf16 matmul noise).
OFFLOAD = (2, 6, 10, 14, 18, 22, 26, 29)
SCHRAUD_SIGMA = 0.06
SCHRAUD_A = float(SCALE * 128.0 * np.log2(np.e))
SCHRAUD_B = float(128.0 * (127.0 - SCHRAUD_SIGMA))

# Tree level at which partial sums stop on-device and go to the host
# (level 2 tiles each cover 8 k-tiles; 4 roots per superblock).
ROOT_LEVEL = 2
N_ROOT_SLOTS = NKT // 2 // (1 << ROOT_LEVEL)

# Matmul free-dim chunks (512 = one psum bank; walrus rejects 2-bank MMs).
QK_FD = 512
PV_FD = 512

_CACHE = {}


def _build():
    nc = bacc.Bacc("TRN2", target_bir_lowering=False, debug=False)
    # Inputs per core (host pre-arranged, bf16):
    #   qt/kt: [head, d, s]  (transposed layout, d on partitions)
    #   vp:    [head, p, t*128+c] where vp[h, p, 128t+c] = V[128t+p, c]
    qt_d = nc.dram_tensor("qt", [HPC, 128, S], BF16, kind="ExternalInput")
    kt_d = nc.dram_tensor("kt", [HPC, 128, S], BF16, kind="ExternalInput")
    vp_d = nc.dram_tensor("vp", [HPC, 128, S], BF16, kind="ExternalInput")
    # Outputs: unnormalized out in [d, q] layout + raw tree roots (host
    # finishes l = partition-sum of roots, then out/l and transpose).
    o_d = nc.dram_tensor("o", [HPC, NSB, 128, SB], FP32, kind="ExternalOutput")
    r_d = nc.dram_tensor(
        "r", [HPC, NSB, N_ROOT_SLOTS, 128, 2 * SB], BF16, kind="ExternalOutput"
    )

    with ExitStack() as ctx:
        tc = ctx.enter_context(tile.TileContext(nc))

        qkv = ctx.enter_context(tc.tile_pool(name="qkv", bufs=2))
        ptp = ctx.enter_context(tc.tile_pool(name="ptp", bufs=3))
        trp = ctx.enter_context(tc.tile_pool(name="trp", bufs=2))
        drainp = ctx.enter_context(tc.tile_pool(name="drainp", bufs=2))

        # PSUM budget: 8 banks of [128, 512 fp32].
        scp = ctx.enter_context(tc.tile_pool(name="scp", bufs=3, space="PSUM"))  # 6 banks
        otp = ctx.enter_context(tc.tile_pool(name="otp", bufs=1, space="PSUM"))  # 2 banks


        for h in range(HPC):
            # Chunked loads so the first QK can start before full tensors land.
            qt_s = qkv.tile([128, S], BF16, name=f"qt{h}", tag="qt")
            kt_s = qkv.tile([128, S], BF16, name=f"kt{h}", tag="kt")
            v_s = qkv.tile([128, S], BF16, name=f"v{h}", tag="v")
            bounds = [0, 256, 1024, 2048, 3072, 4096]
            for ch in range(len(bounds) - 1):
                cs = slice(bounds[ch], bounds[ch + 1])
                nc.sync.dma_start(kt_s[:, cs], kt_d[h][:, cs])
                nc.sync.dma_start(qt_s[:, cs], qt_d[h][:, cs])
                nc.sync.dma_start(v_s[:, cs], vp_d[h][:, cs])

            for sb in range(NSB):
                q0 = sb * SB
                ot = otp.tile([128, SB], FP32, name=f"ot_{h}_{sb}", tag="ot")

                # Binary tree-sum of P_T pair tiles on the DVE (bf16 2x
                # mode, [128, 2048] ops), stopped at ROOT_LEVEL; the roots
                # stream to DRAM and the host finishes the reduction for l.
                tree = {}
                treectr = [0]
                rootctr = [0]

                def feed(t, level=0):
                    while level in tree:
                        prev = tree.pop(level)
                        treectr[0] += 1
                        nt = trp.tile(
                            [128, 2 * SB], BF16,
                            name=f"tr_{h}_{sb}_{level}_{treectr[0]}",
                            tag=f"tree{level}",
                        )
                        nc.vector.tensor_add(nt, prev, t)
                        t = nt
                        level += 1
                        if level == ROOT_LEVEL:
                            nc.sync.dma_start(r_d[h, sb, rootctr[0]], t)
                            rootctr[0] += 1
                            return
                    tree[level] = t

                def qk(sc_dst, j):
                    for qs in range(SB // QK_FD):
                        nc.tensor.matmul(
                            sc_dst[:, qs * QK_FD:(qs + 1) * QK_FD],
                            kt_s[:, j * 128:(j + 1) * 128],
                            qt_s[:, q0 + qs * QK_FD: q0 + (qs + 1) * QK_FD],
                            start=True, stop=True,
                        )

                def pv(j, pt_slice):
                    for qs in range(SB // PV_FD):
                        nc.tensor.matmul(
                            ot[:, qs * PV_FD:(qs + 1) * PV_FD],
                            v_s[:, j * 128:(j + 1) * 128],
                            pt_slice[:, qs * PV_FD:(qs + 1) * PV_FD],
                            start=(j == 0), stop=(j == NKT - 1),
                        )

                # pt pair tiles: [128, 2048] bf16, halves written by exp of
                # k-tiles 2p and 2p+1 (subtile deps let PV consume halves).
                pt_pairs = {}
                pv_queue = []  # (kt_j, pt_slice) in kt order, flushed at lag 3

                for j in range(NKT):
                    sc = scp.tile([128, SB], FP32, name=f"sc_{h}_{sb}_{j}", tag="sc")
                    qk(sc, j)
                    p = j // 2
                    if p not in pt_pairs:
                        pt_pairs[p] = ptp.tile(
                            [128, 2 * SB], BF16, name=f"pt_{h}_{sb}_{p}", tag="pt"
                        )
                    half = pt_pairs[p][:, (j % 2) * SB:(j % 2 + 1) * SB]
                    if j in OFFLOAD:
                        nc.vector.tensor_scalar(
                            half.bitcast(I16), sc, SCHRAUD_A, SCHRAUD_B,
                            mybir.AluOpType.mult, mybir.AluOpType.add,
                        )
                    else:
                        nc.scalar.activation(
                            half, sc, mybir.ActivationFunctionType.Exp, scale=SCALE
                        )
                    if j % 2 == 1:
                        feed(pt_pairs.pop(p))
                    pv_queue.append((j, half))
                    if len(pv_queue) > 3:
                        pv(*pv_queue.pop(0))
                    if j == NKT - 1:
                        for item in pv_queue:
                            pv(*item)
                        pv_queue.clear()

                assert not tree and rootctr[0] == N_ROOT_SLOTS

                # Superblock drain: out psum -> sbuf -> DRAM (frees ot for
                # the next superblock). Copy runs on the scalar engine,
                # which has slack, keeping the DVE queue clear at the
                # superblock boundary.
                ot_sb = drainp.tile([128, SB], FP32, name=f"otsb_{h}_{sb}", tag="otsb")
                nc.scalar.copy(ot_sb, ot)
                nc.sync.dma_start(o_d[h, sb], ot_sb)
    nc.compile()
    return nc


def _prep_inputs(q, k, v):
    bf = ml_dtypes.bfloat16
    in_maps = []
    for c in range(N_CORES):
        hs = slice(c * HPC, (c + 1) * HPC)
        qt = np.transpose(q[:, hs, :], (1, 2, 0)).astype(bf)   # [HPC, D, S]
        kt = np.transpose(k[:, hs, :], (1, 2, 0)).astype(bf)   # [HPC, D, S]
        vh = np.transpose(v[:, hs, :], (1, 0, 2))              # [HPC, S, D]
        vp = np.ascontiguousarray(
            vh.reshape(HPC, S // 128, 128, D).transpose(0, 2, 1, 3)
        ).reshape(HPC, 128, S).astype(bf)
        in_maps.append({"qt": qt, "kt": kt, "vp": vp})
    return in_maps


def kernel(q, k, v, ring_size=None, **_unused):
    q = np.asarray(q, dtype=np.float32).reshape(S, H, D)
    k = np.asarray(k, dtype=np.float32).reshape(S, H, D)
    v = np.asarray(v, dtype=np.float32).reshape(S, H, D)

    in_maps = _prep_inputs(q, k, v)
    if "nc" not in _CACHE:
        _CACHE["nc"] = _build()
    res = run_bass_kernel_spmd(_CACHE["nc"], in_maps, list(range(N_CORES))).results

    out = np.empty((B, S, H, D), np.float32)
    for c in range(N_CORES):
        o = np.asarray(res[c]["o"])                     # [HPC, NSB, 128, SB] fp32
        r = np.asarray(res[c]["r"]).astype(np.float32)  # [HPC,NSB,SLOTS,128,2SB]
        # roots are [128, 2*SB]: two SB-wide halves (k-tile pair layout)
        l = r.sum(axis=(2, 3)).reshape(HPC, NSB, 2, SB).sum(axis=2)  # [HPC,NSB,SB]
        on = o / l[:, :, None, :]                       # normalize per q
        # [HPC, NSB, D, SB] -> [NSB, SB, HPC, D] -> [S, HPC, D]
        out[0, :, c * HPC:(c + 1) * HPC, :] = (
            on.transpose(1, 3, 0, 2).reshape(S, HPC, D)
        )
    return out


# revision 18
# speedup vs baseline: 1.4484x; 1.0101x over previous
"""Trainium2 Bass kernel: ring attention forward == full softmax attention.

The reference's ring decomposition with the sigmoid/logsigmoid LSE merge is
mathematically exact online softmax, so the output equals plain (non-causal)
multi-head attention over the full sequence:

    out[b,q,h,:] = softmax(Q[b,q,h,:] @ K[b,:,h,:].T / sqrt(D)) @ V[b,:,h,:]

Shapes: B=1, S=4096, H=16, D=128, fp32. ring_size only affects the reference's
chunking, not the result, so it is ignored here.

Sharding: 2 heads per NeuronCore (16 heads / 8 cores), fully independent --
no cross-core communication needed (Ulysses-style head sharding).

Device algorithm per head (flash-style, transposed-scores orientation),
per 1024-wide q superblock, k-tiles j = 0..31:

      scores_T[k,q] = K_tile^T-layout @ Q^T-layout   (PE, bf16, psum fp32)
      P_T = exp(scores_T * scale)                    (ACT, FD=1024, bf16 out)
        ... or for kt in OFFLOAD: a one-instruction Schraudolph fast-exp
        on the DVE (int16 bit trick -> bf16) to offload the saturated ACT
      out_T[d,q]  += V_tile^T @ P_T                  (PE, accumulate psum)
      tree-sum of P_T k-tiles                        (DVE bf16 2x, [128,2048]
                                                      pair tiles)
  drain: out_T psum -> sbuf (DVE) -> DRAM; 2 tree roots -> DRAM raw (bf16).

Normalization (out/l) and the [d,q]->[q,d] transpose happen on the host
during unsharding: l = per-q partition-sum of the two roots. This removes
all PE transposes and the psum-resident l machinery, freeing 2 PSUM banks
which triple-buffer the score tiles (PSUM: 3x2 scores + 2 out = 8 banks),
so ACT never stalls on QK and PE runs 2 tiles ahead (PV at lag 2).
"""

import numpy as np
import ml_dtypes
from contextlib import ExitStack

import concourse.bass as bass
import concourse.bacc as bacc
import concourse.mybir as mybir
import concourse.tile as tile
from concourse.bass_utils import run_bass_kernel_spmd

B, S, H, D = 1, 4096, 16, 128
N_CORES = 8
HPC = H // N_CORES          # heads per core
SB = 1024                   # q superblock width
NSB = S // SB
NKT = S // 128              # 32 k-tiles of 128 keys
SCALE = float(1.0 / np.sqrt(D))
BF16 = mybir.dt.bfloat16
FP32 = mybir.dt.float32
I16 = mybir.dt.int16

# k-tiles whose exp runs as a Schraudolph fast-exp on the DVE instead of
# exact exp on the saturated ACT engine. bf16(exp(x*SCALE)) ~=
# bitcast_bf16(int16(x*A + B)); end-to-end rel_rms ~1e-3 per offloaded
# tile pair (host-validated: 6 tiles -> ~6e-3 including bf16 matmul noise).
OFFLOAD = (2, 6, 10, 18, 22, 26)
SCHRAUD_SIGMA = 0.06
SCHRAUD_A = float(SCALE * 128.0 * np.log2(np.e))
SCHRAUD_B = float(128.0 * (127.0 - SCHRAUD_SIGMA))

# Tree level at which partial sums stop on-device and go to the host
# (level 2 tiles each cover 8 k-tiles; 4 roots per superblock).
ROOT_LEVEL = 2
N_ROOT_SLOTS = NKT // 2 // (1 << ROOT_LEVEL)

# Matmul free-dim chunks (512 = one psum bank; walrus rejects 2-bank MMs).
QK_FD = 512
PV_FD = 512

_CACHE = {}


def _build():
    nc = bacc.Bacc("TRN2", target_bir_lowering=False, debug=False)
    # Inputs per core (host pre-arranged, bf16):
    #   qt/kt: [head, d, s]  (transposed layout, d on partitions)
    #   vp:    [head, p, t*128+c] where vp[h, p, 128t+c] = V[128t+p, c]
    qt_d = nc.dram_tensor("qt", [HPC, 128, S], BF16, kind="ExternalInput")
    kt_d = nc.dram_tensor("kt", [HPC, 128, S], BF16, kind="ExternalInput")
    vp_d = nc.dram_tensor("vp", [HPC, 128, S], BF16, kind="ExternalInput")
    # Outputs: unnormalized out in [d, q] layout + raw tree roots (host
    # finishes l = partition-sum of roots, then out/l and transpose).
    o_d = nc.dram_tensor("o", [HPC, NSB, 128, SB], FP32, kind="ExternalOutput")
    r_d = nc.dram_tensor(
        "r", [HPC, NSB, N_ROOT_SLOTS, 128, 2 * SB], BF16, kind="ExternalOutput"
    )

    with ExitStack() as ctx:
        tc = ctx.enter_context(tile.TileContext(nc))

        qkv = ctx.enter_context(tc.tile_pool(name="qkv", bufs=2))
        ptp = ctx.enter_context(tc.tile_pool(name="ptp", bufs=4))
        trp = ctx.enter_context(tc.tile_pool(name="trp", bufs=3))
        drainp = ctx.enter_context(tc.tile_pool(name="drainp", bufs=2))

        # PSUM budget: 8 banks of [128, 512 fp32].
        scp = ctx.enter_context(tc.tile_pool(name="scp", bufs=3, space="PSUM"))  # 6 banks
        otp = ctx.enter_context(tc.tile_pool(name="otp", bufs=1, space="PSUM"))  # 2 banks


        for h in range(HPC):
            # Chunked loads so the first QK can start before full tensors land.
            qt_s = qkv.tile([128, S], BF16, name=f"qt{h}", tag="qt")
            kt_s = qkv.tile([128, S], BF16, name=f"kt{h}", tag="kt")
            v_s = qkv.tile([128, S], BF16, name=f"v{h}", tag="v")
            bounds = [0, 256, 1024, 2048, 3072, 4096]
            for ch in range(len(bounds) - 1):
                cs = slice(bounds[ch], bounds[ch + 1])
                nc.sync.dma_start(kt_s[:, cs], kt_d[h][:, cs])
                nc.sync.dma_start(qt_s[:, cs], qt_d[h][:, cs])
                nc.sync.dma_start(v_s[:, cs], vp_d[h][:, cs])

            for sb in range(NSB):
                q0 = sb * SB
                ot = otp.tile([128, SB], FP32, name=f"ot_{h}_{sb}", tag="ot")

                # Binary tree-sum of P_T pair tiles on the DVE (bf16 2x
                # mode, [128, 2048] ops), stopped at ROOT_LEVEL; the roots
                # stream to DRAM and the host finishes the reduction for l.
                tree = {}
                treectr = [0]
                rootctr = [0]

                def feed(t, level=0):
                    while level in tree:
                        prev = tree.pop(level)
                        treectr[0] += 1
                        nt = trp.tile(
                            [128, 2 * SB], BF16,
                            name=f"tr_{h}_{sb}_{level}_{treectr[0]}",
                            tag=f"tree{level}",
                        )
                        nc.vector.tensor_add(nt, prev, t)
                        t = nt
                        level += 1
                        if level == ROOT_LEVEL:
                            nc.sync.dma_start(r_d[h, sb, rootctr[0]], t)
                            rootctr[0] += 1
                            return
                    tree[level] = t

                def qk(sc_dst, j):
                    for qs in range(SB // QK_FD):
                        nc.tensor.matmul(
                            sc_dst[:, qs * QK_FD:(qs + 1) * QK_FD],
                            kt_s[:, j * 128:(j + 1) * 128],
                            qt_s[:, q0 + qs * QK_FD: q0 + (qs + 1) * QK_FD],
                            start=True, stop=True,
                        )

                def pv(j, pt_slice):
                    for qs in range(SB // PV_FD):
                        nc.tensor.matmul(
                            ot[:, qs * PV_FD:(qs + 1) * PV_FD],
                            v_s[:, j * 128:(j + 1) * 128],
                            pt_slice[:, qs * PV_FD:(qs + 1) * PV_FD],
                            start=(j == 0), stop=(j == NKT - 1),
                        )

                # pt pair tiles: [128, 2048] bf16, halves written by exp of
                # k-tiles 2p and 2p+1 (subtile deps let PV consume halves).
                pt_pairs = {}
                pv_queue = []  # (kt_j, pt_slice) in kt order, flushed at lag 3

                for j in range(NKT):
                    sc = scp.tile([128, SB], FP32, name=f"sc_{h}_{sb}_{j}", tag="sc")
                    qk(sc, j)
                    p = j // 2
                    if p not in pt_pairs:
                        pt_pairs[p] = ptp.tile(
                            [128, 2 * SB], BF16, name=f"pt_{h}_{sb}_{p}", tag="pt"
                        )
                    half = pt_pairs[p][:, (j % 2) * SB:(j % 2 + 1) * SB]
                    if j in OFFLOAD:
                        nc.vector.tensor_scalar(
                            half.bitcast(I16), sc, SCHRAUD_A, SCHRAUD_B,
                            mybir.AluOpType.mult, mybir.AluOpType.add,
                        )
                    else:
                        nc.scalar.activation(
                            half, sc, mybir.ActivationFunctionType.Exp, scale=SCALE
                        )
                    if j % 2 == 1:
                        feed(pt_pairs.pop(p))
                    pv_queue.append((j, half))
                    if len(pv_queue) > 3:
                        pv(*pv_queue.pop(0))
                    if j == NKT - 1:
                        for item in pv_queue:
                            pv(*item)
                        pv_queue.clear()

                assert not tree and rootctr[0] == N_ROOT_SLOTS

                # Superblock drain: out psum -> sbuf -> DRAM (frees ot for
                # the next superblock). Copy runs on the scalar engine,
                # which has slack, keeping the DVE queue clear at the
                # superblock boundary.
                ot_sb = drainp.tile([128, SB], FP32, name=f"otsb_{h}_{sb}", tag="otsb")
                nc.vector.tensor_copy(ot_sb, ot)
                nc.sync.dma_start(o_d[h, sb], ot_sb)
    nc.compile()
    return nc


def _prep_inputs(q, k, v):
    bf = ml_dtypes.bfloat16
    in_maps = []
    for c in range(N_CORES):
        hs = slice(c * HPC, (c + 1) * HPC)
        qt = np.transpose(q[:, hs, :], (1, 2, 0)).astype(bf)   # [HPC, D, S]
        kt = np.transpose(k[:, hs, :], (1, 2, 0)).astype(bf)   # [HPC, D, S]
        vh = np.transpose(v[:, hs, :], (1, 0, 2))              # [HPC, S, D]
        vp = np.ascontiguousarray(
            vh.reshape(HPC, S // 128, 128, D).transpose(0, 2, 1, 3)
        ).reshape(HPC, 128, S).astype(bf)
        in_maps.append({"qt": qt, "kt": kt, "vp": vp})
    return in_maps


def kernel(q, k, v, ring_size=None, **_unused):
    q = np.asarray(q, dtype=np.float32).reshape(S, H, D)
    k = np.asarray(k, dtype=np.float32).reshape(S, H, D)
    v = np.asarray(v, dtype=np.float32).reshape(S, H, D)

    in_maps = _prep_inputs(q, k, v)
    if "nc" not in _CACHE:
        _CACHE["nc"] = _build()
    res = run_bass_kernel_spmd(_CACHE["nc"], in_maps, list(range(N_CORES))).results

    out = np.empty((B, S, H, D), np.float32)
    for c in range(N_CORES):
        o = np.asarray(res[c]["o"])                     # [HPC, NSB, 128, SB] fp32
        r = np.asarray(res[c]["r"]).astype(np.float32)  # [HPC,NSB,SLOTS,128,2SB]
        # roots are [128, 2*SB]: two SB-wide halves (k-tile pair layout)
        l = r.sum(axis=(2, 3)).reshape(HPC, NSB, 2, SB).sum(axis=2)  # [HPC,NSB,SB]
        on = o / l[:, :, None, :]                       # normalize per q
        # [HPC, NSB, D, SB] -> [NSB, SB, HPC, D] -> [S, HPC, D]
        out[0, :, c * HPC:(c + 1) * HPC, :] = (
            on.transpose(1, 3, 0, 2).reshape(S, HPC, D)
        )
    return out
